# revision 9
# baseline (speedup 1.0000x reference)
"""MixedEmbeddingV2 Trainium2 kernel.

out[b, s, :] = emb_weight[x[b, s], :] * col_scale
  col_scale[j] = sum_i weights[i] * [j < dims_i],  dims = (192, 384, 576, 768)

Sharding: token-parallel across 8 cores (batch row b -> core b), table
replicated per core. No collectives. Per core: 16 indirect-DMA row gathers
of [128, 768] f32, DVE column-scale multiply, contiguous write-back.

Raw Bass (not Tile): the DVE TensorTensor encoding on TRN2 rejects multiple
attached sync waits, so all cross-engine sync is standalone wait_ge
instructions with one semaphore per producer stream.
"""

import numpy as np

VOCAB = 50257
D = 768
B, S = 8, 2048
N_CORES = 8
TOK = (B * S) // N_CORES  # 2048 tokens per core
NT = TOK // 128           # 16 gather tiles per core
DIMS = (192, 384, 576, 768)

_cache = {}


def _build_nc(R=1):
    # R = benchmark repeat count: the pipeline body runs R times inside one
    # NEFF (R>1 reuses tiles with slot-recycle waits). Grading uses R=1.
    import concourse.bass as bass
    import concourse.mybir as mybir
    from contextlib import ExitStack

    f32 = mybir.dt.float32
    i32 = mybir.dt.int32

    nc = bass.Bass()
    x_h = nc.declare_dram_parameter("x_idx", [128, NT], i32, isOutput=False)
    s_h = nc.declare_dram_parameter("col_scale", [128, D], f32, isOutput=False)
    t_h = nc.declare_dram_parameter("emb", [VOCAB, D], f32, isOutput=False)
    o_h = nc.declare_dram_parameter("out", [TOK, D], f32, isOutput=True)

    with ExitStack() as es:
        idx = es.enter_context(nc.sbuf_tensor("idx", [128, NT], i32))
        scale = es.enter_context(nc.sbuf_tensor("scale", [128, D], f32))
        gts = [
            es.enter_context(nc.sbuf_tensor(f"gt{g}", [128, D], f32))
            for g in range(NT)
        ]
        i_sem = es.enter_context(nc.semaphore("i_sem"))
        s_sem = es.enter_context(nc.semaphore("s_sem"))
        g_sems = [
            es.enter_context(nc.semaphore(f"g_sem{g}")) for g in range(NT)
        ]
        m_sem = es.enter_context(nc.semaphore("m_sem"))
        o_sem = es.enter_context(nc.semaphore("o_sem"))

        with nc.Block() as block:

            @block.sync
            def _(sync: bass.BassEngine):
                sync.dma_start(out=idx[:], in_=x_h[:]).then_inc(i_sem, 16)
                sync.dma_start(out=scale[:], in_=s_h[:]).then_inc(s_sem, 16)
                # end-of-kernel drain: all output stores landed
                sync.wait_ge(o_sem, 16 * NT * R)

            @block.gpsimd
            def _(gp: bass.BassEngine):
                gp.wait_ge(i_sem, 16)
                for r in range(R):
                    for g in range(NT):
                        if r > 0:
                            # slot recycle: round r-1's store of this tile
                            # must have drained before regathering into it
                            gp.wait_ge(o_sem, 16 * (NT * (r - 1) + g + 1))
                        gp.indirect_dma_start(
                            out=gts[g][:],
                            out_offset=None,
                            in_=t_h[:],
                            in_offset=bass.IndirectOffsetOnAxis(
                                ap=idx[:, g : g + 1], axis=0
                            ),
                        ).then_inc(g_sems[g], 16)

            @block.vector
            def _(v: bass.BassEngine):
                v.wait_ge(s_sem, 16)
                for r in range(R):
                    for g in range(NT):
                        v.wait_ge(g_sems[g], 16 * (r + 1))
                        v.tensor_mul(
                            out=gts[g][:], in0=gts[g][:], in1=scale[:]
                        ).then_inc(m_sem, 1)

            @block.scalar
            def _(sc: bass.BassEngine):
                for r in range(R):
                    for g in range(NT):
                        sc.wait_ge(m_sem, NT * r + g + 1)
                        sc.dma_start(
                            out=o_h[g * 128 : (g + 1) * 128, :], in_=gts[g][:]
                        ).then_inc(o_sem, 16)

    return nc


def _get_nc(R=1):
    key = ("nc", R)
    if key not in _cache:
        _cache[key] = _build_nc(R)
    return _cache[key]


def _make_in_maps(x, weights, emb_weight):
    weights = np.asarray(weights, dtype=np.float32)
    emb = np.ascontiguousarray(np.asarray(emb_weight, dtype=np.float32))

    col = np.arange(D)
    mask = (col[None, :] < np.asarray(DIMS)[:, None]).astype(np.float32)
    col_scale = (weights @ mask).astype(np.float32)  # [D]
    scale_bcast = np.ascontiguousarray(np.broadcast_to(col_scale, (128, D)))

    x32 = np.asarray(x).reshape(N_CORES, TOK).astype(np.int32)
    in_maps = []
    for c in range(N_CORES):
        # SBUF idx tile [p, g] holds token g*128+p of this core's shard.
        xi = np.ascontiguousarray(x32[c].reshape(NT, 128).T)
        in_maps.append({"x_idx": xi, "col_scale": scale_bcast, "emb": emb})
    return in_maps


def _run(x, weights, emb_weight, **spmd_kwargs):
    from concourse.bass_utils import run_bass_kernel_spmd

    in_maps = _make_in_maps(x, weights, emb_weight)
    nc = _get_nc()
    res = run_bass_kernel_spmd(nc, in_maps, list(range(N_CORES)), **spmd_kwargs)
    out = np.stack([res.results[c]["out"] for c in range(N_CORES)], axis=0)
    return out.reshape(B, S, D), res


def kernel(x, weights, emb_weight):
    out, _ = _run(x, weights, emb_weight)
    return out
